# revision 1
# baseline (speedup 1.0000x reference)
"""FaceAttnProcessor Trainium2 kernel.

Sharding: 8 cores = batch(2) x row-slices(4 x 256 rows). Each core computes
its 256 query rows end-to-end (self-attn with redundant K/V over the full
1040-token sequence, GEGLU FF, cross-attn against the 77 text tokens).
No collectives; the host scatters inputs and gathers the 8 row-slices.

Dataflow: activations kept in natural [rows, C] fp32 for LN / softmax-stats /
residuals, and transposed [C, rows] float32r for matmuls (PE transposes, cast
fused into the PSUM->SBUF copyback). All matmuls run in float32r (tf32-class:
~1.6e-4 rel err, 4x the fp32 PE rate at free-dim >= 256). Scores are computed
pre-transposed (S^T = K_h^T.T @ Q_h^T); softmax needs no max-subtraction
(|S| <~ 2.5 for these normed inputs / 0.02-scale weights); row-sums via
ones-vector matmuls and the 1/rowsum applied via a DRAM-roundtrip
partition-broadcast of the reciprocals.
"""
import numpy as np
from contextlib import ExitStack

import concourse.bass as bass
import concourse.tile as tile
import concourse.mybir as mybir
from concourse import bacc
from concourse.bass_utils import run_bass_kernel_spmd
from concourse.masks import make_identity

F32 = mybir.dt.float32
F32R = mybir.dt.float32r
AFT = mybir.ActivationFunctionType

P = 128
B, N, C, L = 2, 1024, 768, 93
NT, NF = 77, 16            # text / face tokens
NTP = 80                   # text tokens padded (fp32r needs even free dims)
NC_ = 1040                 # N + NF combined sequence
R = 256                    # query rows per core
H, D = 12, 64              # heads, head dim
HP = 6                     # head pairs
INNER = 3072
KC = 6                     # C // 128
EPS = 1e-5

_cache = {}


def build():
    nc = bacc.Bacc("TRN2", target_bir_lowering=False, debug=False, num_devices=8)

    x_own_d = nc.dram_tensor("x_own", [R, C], F32, kind="ExternalInput")
    x_full_d = nc.dram_tensor("x_full", [N, C], F32, kind="ExternalInput")
    ehs_d = nc.dram_tensor("ehs", [L, C], F32, kind="ExternalInput")
    wq_d = nc.dram_tensor("sa_wq", [C, C], F32, kind="ExternalInput")
    wk_d = nc.dram_tensor("sa_wk", [C, C], F32, kind="ExternalInput")
    wv_d = nc.dram_tensor("sa_wv", [C, C], F32, kind="ExternalInput")
    wo_d = nc.dram_tensor("sa_wo", [C, C], F32, kind="ExternalInput")
    wob_d = nc.dram_tensor("sa_wo_b", [C], F32, kind="ExternalInput")
    ln1g_d = nc.dram_tensor("ln1_g", [C], F32, kind="ExternalInput")
    ln1b_d = nc.dram_tensor("ln1_b", [C], F32, kind="ExternalInput")
    ln2g_d = nc.dram_tensor("ln2_g", [C], F32, kind="ExternalInput")
    ln2b_d = nc.dram_tensor("ln2_b", [C], F32, kind="ExternalInput")
    ffg_d = nc.dram_tensor("ff_ln_g", [C], F32, kind="ExternalInput")
    ffb_d = nc.dram_tensor("ff_ln_b", [C], F32, kind="ExternalInput")
    w1_d = nc.dram_tensor("ff_w1", [C, 2 * INNER], F32, kind="ExternalInput")
    w2_d = nc.dram_tensor("ff_w2", [INNER, C], F32, kind="ExternalInput")
    aa_d = nc.dram_tensor("alpha_attn", [1, 1], F32, kind="ExternalInput")
    ad_d = nc.dram_tensor("alpha_dense", [1, 1], F32, kind="ExternalInput")
    cq_d = nc.dram_tensor("ca_wq", [C, C], F32, kind="ExternalInput")
    ck_d = nc.dram_tensor("ca_wk", [C, C], F32, kind="ExternalInput")
    cv_d = nc.dram_tensor("ca_wv", [C, C], F32, kind="ExternalInput")
    co_d = nc.dram_tensor("ca_wo", [C, C], F32, kind="ExternalInput")
    cob_d = nc.dram_tensor("ca_wo_b", [C], F32, kind="ExternalInput")
    out_d = nc.dram_tensor("out_own", [R, C], F32, kind="ExternalOutput")

    with tile.TileContext(nc) as tc, ExitStack() as ctx:
        consts = ctx.enter_context(tc.tile_pool(name="consts", bufs=1))
        acts = ctx.enter_context(tc.tile_pool(name="acts", bufs=1))
        tmp = ctx.enter_context(tc.tile_pool(name="tmp", bufs=2))
        dram = ctx.enter_context(tc.tile_pool(name="dram", bufs=1, space="DRAM"))

        # ---------------- constants ----------------
        ident = consts.tile([P, P], F32)
        make_identity(nc, ident[:])
        ones_r = consts.tile([P, 1], F32R)
        nc.vector.memset(ones_r[:].bitcast(F32), 1.0)
        eps_t = consts.tile([P, 1], F32)
        nc.vector.memset(eps_t[:], EPS)

        def vec_T(d):   # per-channel vector in ^T form [128, 6]
            t = consts.tile([P, KC], F32, tag=f"vt_{d.name}")
            nc.sync.dma_start(t[:], d.rearrange("(ko p) -> p ko", p=P))
            return t

        def vec_bc(d):  # per-channel vector broadcast across partitions
            t = consts.tile([P, C], F32, tag=f"vb_{d.name}")
            nc.sync.dma_start(t[:], d[None, :].to_broadcast([P, C]))
            return t

        g1T, b1T = vec_T(ln1g_d), vec_T(ln1b_d)


        # ---------------- helpers ----------------
        def ln_stats(x_ap, p):
            """Normalized (x-m)/std for natural tile slice x_ap [p, 768] fp32.
            var = E[x^2] - m^2 (shared junk buffer for the squared output)."""
            junk = tmp.tile([P, C], F32, tag="ln_xc")
            vsum = tmp.tile([P, 1], F32, tag="ln_vsum")
            nc.scalar.activation(junk[:p], x_ap, AFT.Square, accum_out=vsum[:p])
            mean = tmp.tile([P, 1], F32, tag="ln_mean")
            nc.vector.reduce_sum(mean[:p], x_ap, axis=mybir.AxisListType.X)
            nc.vector.tensor_scalar_mul(mean[:p], mean[:p], 1.0 / C)
            m2 = tmp.tile([P, 1], F32, tag="ln_m2")
            nc.vector.tensor_mul(m2[:p], mean[:p], mean[:p])
            var = tmp.tile([P, 1], F32, tag="ln_var")
            nc.vector.tensor_scalar_mul(var[:p], vsum[:p], 1.0 / C)
            nc.vector.tensor_sub(var[:p], var[:p], m2[:p])
            std = tmp.tile([P, 1], F32, tag="ln_std")
            nc.scalar.activation(std[:p], var[:p], AFT.Sqrt, bias=eps_t[:p, 0:1])
            rstd = tmp.tile([P, 1], F32, tag="ln_rstd")
            nc.vector.reciprocal(rstd[:p], std[:p])
            xn = tmp.tile([P, C], F32, tag="ln_xn")
            nc.vector.tensor_scalar(xn[:p], x_ap, mean[:p], rstd[:p],
                                    mybir.AluOpType.subtract, mybir.AluOpType.mult)
            return xn

        def transpose_gb(ps_t, xn, p, dst, col, gT, bT, flip=0):
            """PE-transpose xn [p,768] into dst[:, k, col:col+p] (f32r), applying
            per-channel gain/bias in the copyback (channels on partitions)."""
            for k in range(KC):
                pt = ps_t.tile([P, P], F32, tag="tp")
                nc.tensor.transpose(pt[:, 0:p], xn[:p, bass.ts(k, P)], ident[:p, :p])
                if (k + flip) % 2 == 0:
                    nc.vector.tensor_scalar(
                        dst[:, k, col:col + p], pt[:, 0:p],
                        gT[:, k:k + 1], bT[:, k:k + 1],
                        mybir.AluOpType.mult, mybir.AluOpType.add)
                else:
                    nc.scalar.activation(
                        dst[:, k, col:col + p], pt[:, 0:p],
                        AFT.Identity, bias=bT[:, k:k + 1], scale=gT[:, k:k + 1])

        def transpose_plain(ps_t, src_ap, p, dst_ap, scale=None):
            """PE-transpose src [p, 128] sbuf fp32 -> dst [128, p] (any dtype)."""
            pt = ps_t.tile([P, P], F32, tag="tp")
            nc.tensor.transpose(pt[:, 0:p], src_ap, ident[:p, :p])
            if scale is None:
                nc.vector.tensor_copy(dst_ap, pt[:, 0:p])
            else:
                nc.scalar.activation(dst_ap, pt[:, 0:p], AFT.Copy, scale=scale)


        # ---------------- persistent activations ----------------
        xo = acts.tile([P, 2, C], F32, tag="xo")
        nc.sync.dma_start(xo[:], x_own_d.rearrange("(rc p) c -> p rc c", p=P))
        # bulk per-channel broadcasts emitted after the latency-critical loads
        ffgT, ffbT = vec_T(ffg_d), vec_T(ffb_d)
        g2B, b2B = vec_bc(ln2g_d), vec_bc(ln2b_d)
        cobB = vec_bc(cob_d)
        wobT = consts.tile([P, C], F32, tag="wobT")
        nc.sync.dma_start(wobT[:], wob_d[None, :].to_broadcast([P, C]))
        x1 = acts.tile([P, 2, C], F32, tag="x1")
        x2 = acts.tile([P, 2, C], F32, tag="x2")
        textT = acts.tile([P, KC, NTP], F32R, tag="textT")
        KcaT = acts.tile([P, KC, NTP], F32R, tag="KcaT")
        Vca = acts.tile([NTP, C], F32R, tag="Vca")

        with tc.tile_pool(name="saout", bufs=1) as saout:
            attnUT = saout.tile([P, HP, R], F32R, tag="attnUT")  # pair-form
            srec = dram.tile([HP, 2 * R], F32)

            with tc.tile_pool(name="sa", bufs=1) as sa:
                QT = sa.tile([P, KC, R], F32R, tag="QT")
                KT = sa.tile([P, KC, NC_], F32R, tag="KT")
                V = sa.tile([P, 9, C], F32R, tag="V")

                # ---- LN1 -> comb^T / q_src^T, then QKV (pre pools close after) ----
                with tc.tile_pool(name="pre", bufs=1) as pre, \
                     tc.tile_pool(name="prexf", bufs=4) as prexf, \
                     tc.tile_pool(name="ps_t0", bufs=2, space="PSUM") as ps_t0, \
                     tc.tile_pool(name="wstr", bufs=2) as wstr:

                    cT = pre.tile([P, KC, NC_], F32R, tag="cT")
                    qsT = pre.tile([P, KC, R], F32R, tag="qsT")
                    text = pre.tile([NT, C], F32, tag="text")
                    nc.sync.dma_start(text[:], ehs_d[0:NT, :])
                    face = pre.tile([NF, C], F32, tag="face")
                    nc.sync.dma_start(face[:], ehs_d[NT:L, :])

                    # warmup transpose: first real transpose carries one sem wait
                    ptw = ps_t0.tile([P, P], F32, tag="tp")
                    nc.tensor.transpose(ptw[:], ident[:], ident[:])

                    # text^T early (independent of x) to fill PE during LN
                    nc.vector.memset(textT[:, :, NT:NTP].bitcast(F32), 0.0)
                    for k in range(KC):
                        transpose_plain(ps_t0, text[0:NT, bass.ts(k, P)], NT,
                                        textT[:, k, 0:NT])

                    for rc in range(8):
                        xf = prexf.tile([P, C], F32, tag="xf")
                        nc.sync.dma_start(xf[:], x_full_d[rc * P:(rc + 1) * P, :])
                        xn = ln_stats(xf[:, :], P)
                        transpose_gb(ps_t0, xn, P, cT, rc * P, g1T, b1T, rc)
                    fn = ln_stats(face[:], NF)
                    transpose_gb(ps_t0, fn, NF, cT, N, g1T, b1T)
                    for rc in range(2):
                        xn = ln_stats(xo[:, rc, :], P)
                        transpose_gb(ps_t0, xn, P, qsT, rc * P, g1T, b1T, rc)

                    def load_w_chunk(d, f0, fw, tag="wch"):
                        t = wstr.tile([P, KC, 512], F32R, tag=tag, name="wchunk")
                        nc.gpsimd.dma_start(
                            t[:, :, 0:fw],
                            d[:, f0:f0 + fw].rearrange("(ko p) f -> p ko f", p=P))
                        return t

                    with tc.tile_pool(name="ps_qkv", bufs=3, space="PSUM") as ps_qkv:
                        # V natural (Form 1)
                        for f0, fw in ((0, 512), (512, 256)):
                            wvc = load_w_chunk(wv_d, f0, fw)
                            for rc in range(9):
                                p = P if rc < 8 else NF
                                pv = ps_qkv.tile([P, 512], F32, tag="pqkv", name="pv")
                                for k in range(KC):
                                    nc.tensor.matmul(pv[:p, 0:fw],
                                                     cT[:, k, rc * P:rc * P + p],
                                                     wvc[:, k, 0:fw],
                                                     start=(k == 0), stop=(k == KC - 1))
                                nc.vector.tensor_copy(V[:p, rc, f0:f0 + fw],
                                                      pv[:p, 0:fw])
                        # Q^T (Form 2), 1/sqrt(d) folded into copyback
                        for fc0, fcw in ((0, 512), (512, 256)):
                            wqc = load_w_chunk(wq_d, fc0, fcw)
                            for fi in range(fcw // P):
                                f = fc0 // P + fi
                                pq = ps_qkv.tile([P, 512], F32, tag="pqkv", name="pq")
                                for k in range(KC):
                                    nc.tensor.matmul(pq[:, 0:R],
                                                     wqc[:, k, bass.ts(fi, P)],
                                                     qsT[:, k, :],
                                                     start=(k == 0), stop=(k == KC - 1))
                                nc.scalar.activation(QT[:, f, :], pq[:, 0:R],
                                                     AFT.Copy, scale=0.125)
                        # K^T (Form 2)
                        for fc0, fcw in ((0, 512), (512, 256)):
                            wkc = load_w_chunk(wk_d, fc0, fcw)
                            for fi in range(fcw // P):
                                f = fc0 // P + fi
                                for j0, jw in ((0, 512), (512, 512), (1024, NF)):
                                    pk = ps_qkv.tile([P, 512], F32, tag="pqkv",
                                                     name="pk")
                                    for k in range(KC):
                                        nc.tensor.matmul(pk[:, 0:jw],
                                                         wkc[:, k, bass.ts(fi, P)],
                                                         cT[:, k, j0:j0 + jw],
                                                         start=(k == 0),
                                                         stop=(k == KC - 1))
                                    nc.vector.tensor_copy(KT[:, f, j0:j0 + jw],
                                                          pk[:, 0:jw])
                        # CA K^T (Form 2) and V_ca (Form 1): only need text
                        for fc0, fcw in ((0, 512), (512, 256)):
                            ckc = load_w_chunk(ck_d, fc0, fcw)
                            for fi in range(fcw // P):
                                f = fc0 // P + fi
                                pk = ps_qkv.tile([P, 512], F32, tag="pqkv",
                                                 name="pck")
                                for k in range(KC):
                                    nc.tensor.matmul(pk[:, 0:NTP],
                                                     ckc[:, k, bass.ts(fi, P)],
                                                     textT[:, k, :],
                                                     start=(k == 0),
                                                     stop=(k == KC - 1))
                                nc.vector.tensor_copy(KcaT[:, f, :], pk[:, 0:NTP])
                        for f0, fw in ((0, 512), (512, 256)):
                            cvc = load_w_chunk(cv_d, f0, fw)
                            pv = ps_qkv.tile([P, 512], F32, tag="pqkv", name="pcv")
                            for k in range(KC):
                                nc.tensor.matmul(pv[0:NTP, 0:fw], textT[:, k, :],
                                                 cvc[:, k, 0:fw],
                                                 start=(k == 0), stop=(k == KC - 1))
                            nc.vector.tensor_copy(Vca[:, f0:f0 + fw],
                                                  pv[0:NTP, 0:fw])

                # tanh(alpha_*) -> [128,1]; emitted after LN so the DVE
                # startup path isn't serialized behind this DMA chain
                alo = consts.tile([1, 2], F32)
                nc.sync.dma_start(alo[:, 0:1], aa_d[:])
                nc.sync.dma_start(alo[:, 1:2], ad_d[:])
                th = consts.tile([1, 2], F32)
                nc.scalar.activation(th[:], alo[:], AFT.Tanh)
                tanh_dr = dram.tile([1, 2], F32)
                nc.sync.dma_start(tanh_dr[:], th[:])
                tA = consts.tile([P, 1], F32, tag="tA")
                nc.sync.dma_start(tA[:], tanh_dr[0:1, 0:1].to_broadcast([P, 1]))
                tD = consts.tile([P, 1], F32, tag="tD")
                nc.sync.dma_start(tD[:], tanh_dr[0:1, 1:2].to_broadcast([P, 1]))
                # wobT := tanh(aa) * wo_b, scaled in place
                nc.vector.tensor_scalar_mul(wobT[:], wobT[:], tA[:, 0:1])

                # ---- self-attention, per head pair ----
                with tc.tile_pool(name="ps_sc", bufs=3, space="PSUM") as ps_sc, \
                     tc.tile_pool(name="ps_ss", bufs=1, space="PSUM") as ps_ss, \
                     tc.tile_pool(name="ps_av", bufs=2, space="PSUM") as ps_av, \
                     tc.tile_pool(name="expp", bufs=14) as expp:
                    for hp in range(HP):
                        pss = ps_ss.tile([1, 2 * R], F32, tag="pss")
                        # pass 1: scores + exp + rowsum; the reciprocal DMA
                        # roundtrip launches BEFORE the attnV matmuls so the
                        # division inputs are ready when attnV drains.
                        ests = []
                        for rc in range(9):
                            p = P if rc < 8 else NF
                            est = expp.tile([P, 2, R], F32R, tag="est",
                                            name=f"est{hp}_{rc}")
                            ests.append(est)
                            for h01 in range(2):
                                b0 = h01 * D
                                psc = ps_sc.tile([P, R], F32, tag="psc")
                                nc.tensor.matmul(psc[0:p, :],
                                                 KT[b0:b0 + D, hp, rc * P:rc * P + p],
                                                 QT[b0:b0 + D, hp, :],
                                                 start=True, stop=True)
                                nc.scalar.activation(est[0:p, h01, :], psc[0:p, :],
                                                     AFT.Exp)
                            nc.tensor.matmul(pss[:], ones_r[0:p, :],
                                             est[0:p, :, :].rearrange(
                                                 "p a b -> p (a b)"),
                                             start=(rc == 0), stop=(rc == 8))
                        rs = tmp.tile([1, 2 * R], F32, tag="rs")
                        nc.vector.reciprocal(rs[:], pss[:])
                        nc.sync.dma_start(srec[hp:hp + 1, :], rs[:])
                        rbcA = tmp.tile([D, R], F32, tag="rbcA")
                        nc.sync.dma_start(rbcA[:],
                                          srec[hp:hp + 1, 0:R].to_broadcast([D, R]))
                        rbcB = tmp.tile([D, R], F32, tag="rbcB")
                        nc.sync.dma_start(rbcB[:],
                                          srec[hp:hp + 1, R:2 * R].to_broadcast([D, R]))
                        # pass 2: attnV accumulation, then divide
                        pavA = ps_av.tile([D, R], F32, tag="pavA")
                        pavB = ps_av.tile([D, R], F32, tag="pavB")
                        for rc in range(9):
                            p = P if rc < 8 else NF
                            nc.tensor.matmul(pavA[:],
                                             V[0:p, rc, (2 * hp) * D:(2 * hp + 1) * D],
                                             ests[rc][0:p, 0, :],
                                             start=(rc == 0), stop=(rc == 8))
                            nc.tensor.matmul(pavB[:],
                                             V[0:p, rc,
                                               (2 * hp + 1) * D:(2 * hp + 2) * D],
                                             ests[rc][0:p, 1, :],
                                             start=(rc == 0), stop=(rc == 8))
                        nc.vector.tensor_mul(attnUT[0:D, hp, :], pavA[:], rbcA[:])
                        ost = tmp.tile([D, R], F32R, tag="ost")
                        nc.vector.tensor_mul(ost[:], pavB[:], rbcB[:])
                        # partition-shift the odd head into rows 64:128 via DMA
                        nc.sync.dma_start(attnUT[D:P, hp, :], ost[:])

            # ---- O-proj + gated residual -> x1 ----
            with tc.tile_pool(name="wstr2", bufs=2) as wstr2, \
                 tc.tile_pool(name="ps_pr", bufs=2, space="PSUM") as ps_pr:
                for f0, fw in ((0, 512), (512, 256)):
                    woc = wstr2.tile([P, HP, 512], F32R, tag="woc")
                    nc.gpsimd.dma_start(
                        woc[:, :, 0:fw],
                        wo_d[:, f0:f0 + fw].rearrange("(hp p) f -> p hp f", p=P))
                    for qc in range(2):
                        po = ps_pr.tile([P, 512], F32, tag="po")
                        for hp in range(HP):
                            nc.tensor.matmul(po[:, 0:fw],
                                             attnUT[:, hp, bass.ts(qc, P)],
                                             woc[:, hp, 0:fw],
                                             start=(hp == 0), stop=(hp == HP - 1))
                        xs = x1[:, qc, f0:f0 + fw]
                        nc.vector.tensor_scalar_mul(xs, po[:, 0:fw], tA[:, 0:1])
                        nc.vector.tensor_add(xs, xs, wobT[:, f0:f0 + fw])
                        nc.vector.tensor_add(xs, xs, xo[:, qc, f0:f0 + fw])

        # ---------------- FF ----------------
        with tc.tile_pool(name="ffp", bufs=1) as ffp, \
             tc.tile_pool(name="ps_tf", bufs=2, space="PSUM") as ps_tf:
            hT = ffp.tile([P, KC, R], F32R, tag="hT")
            for rc in range(2):
                xn = ln_stats(x1[:, rc, :], P)
                z = tmp.tile([P, C], F32, tag="ln_xn", name="z")
                nc.vector.tensor_mul(z[:], xn[:], g2B[:])
                nc.vector.tensor_add(z[:], z[:], b2B[:])
                zn = ln_stats(z[:], P)
                transpose_gb(ps_tf, zn, P, hT, rc * P, ffgT, ffbT, rc)

            actT = ffp.tile([P, 24, R], F32R, tag="actT")
            wff2_cm = tc.tile_pool(name="wff2", bufs=4)
            wff2 = wff2_cm.__enter__()
            w2cs = []

            def load_w2_quarter(qb):
                # interleaved with the FF1 stream: the previous 4.7MB
                # half-loads (13us each) monopolized the DMA engines and
                # stalled the O-projection's small attnUT shift DMA by 25us
                w2c = wff2.tile([P, KC, C], F32R, tag="w2c", name=f"w2c{qb}")
                nc.gpsimd.dma_start(
                    w2c[:], w2_d[qb * C:(qb + 1) * C, :].rearrange(
                        "(ko p) f -> p ko f", p=P))
                w2cs.append(w2c)
            with tc.tile_pool(name="wff1", bufs=3) as wff1, \
                 tc.tile_pool(name="ps_h1", bufs=2, space="PSUM") as ps_h1:
                for fc in range(12):
                    if fc % 3 == 0:
                        load_w2_quarter(fc // 3)
                    w1a = wff1.tile([P, KC, 256], F32R, tag="w1a")
                    nc.gpsimd.dma_start(
                        w1a[:], w1_d[:, fc * 256:(fc + 1) * 256].rearrange(
                            "(ko p) f -> p ko f", p=P))
                    w1g = wff1.tile([P, KC, 256], F32R, tag="w1g")
                    nc.gpsimd.dma_start(
                        w1g[:],
                        w1_d[:, INNER + fc * 256:INNER + (fc + 1) * 256].rearrange(
                            "(ko p) f -> p ko f", p=P))
                    for fi in range(2):
                        ft = fc * 2 + fi
                        pa = ps_h1.tile([P, R], F32, tag="pa")
                        pg = ps_h1.tile([P, R], F32, tag="pg")
                        for k in range(KC):
                            nc.tensor.matmul(pa[:], w1a[:, k, bass.ts(fi, P)],
                                             hT[:, k, :],
                                             start=(k == 0), stop=(k == KC - 1))
                        for k in range(KC):
                            nc.tensor.matmul(pg[:], w1g[:, k, bass.ts(fi, P)],
                                             hT[:, k, :],
                                             start=(k == 0), stop=(k == KC - 1))
                        gl = tmp.tile([P, R], F32, tag="gl")
                        nc.scalar.activation(gl[:], pg[:], AFT.Gelu)
                        nc.vector.tensor_mul(actT[:, ft, :], pa[:], gl[:])

            ffT = ffp.tile([P, KC, R], F32, tag="ffT")
            with tc.tile_pool(name="ps_f2", bufs=6, space="PSUM") as ps_f2:
                pfs = []
                for f in range(KC):
                    pf = ps_f2.tile([P, R], F32, tag="pf", name=f"pf{f}")
                    pfs.append(pf)
                for qb in range(4):   # k-quarters of the 3072 contraction
                    for f in range(KC):
                        for k in range(KC):
                            nc.tensor.matmul(pfs[f][:], w2cs[qb][:, k, bass.ts(f, P)],
                                             actT[:, qb * KC + k, :],
                                             start=(qb == 0 and k == 0),
                                             stop=(qb == 3 and k == KC - 1))
                for f in range(KC):
                    nc.vector.tensor_copy(ffT[:, f, :], pfs[f][:])
            wff2_cm.__exit__(None, None, None)

            # x2 = x1 + tanh(ad) * ff
            for qc in range(2):
                for k in range(KC):
                    pt = ps_tf.tile([P, P], F32, tag="tp")
                    nc.tensor.transpose(pt[:], ffT[:, k, bass.ts(qc, P)], ident[:])
                    t2 = tmp.tile([P, P], F32, tag="gl")
                    nc.vector.tensor_scalar_mul(t2[:], pt[:], tD[:, 0:1])
                    nc.vector.tensor_add(x2[:, qc, bass.ts(k, P)], t2[:],
                                         x1[:, qc, bass.ts(k, P)])

        # ---------------- cross-attention ----------------
        with tc.tile_pool(name="cap", bufs=1) as cap, \
             tc.tile_pool(name="ps_tc", bufs=2, space="PSUM") as ps_tc:
            x2T = cap.tile([P, KC, R], F32R, tag="x2T")
            for k in range(KC):
                for qc in range(2):
                    transpose_plain(ps_tc, x2[:, qc, bass.ts(k, P)], P,
                                    x2T[:, k, bass.ts(qc, P)])

            qcaT = cap.tile([P, KC, R], F32R, tag="qcaT")
            with tc.tile_pool(name="wstr3", bufs=2) as wstr3, \
                 tc.tile_pool(name="ps_ca", bufs=3, space="PSUM") as ps_ca:
                for fc0, fcw in ((0, 512), (512, 256)):
                    cqc = wstr3.tile([P, KC, 512], F32R, tag="cwq", name="cqc")
                    nc.gpsimd.dma_start(
                        cqc[:, :, 0:fcw],
                        cq_d[:, fc0:fc0 + fcw].rearrange("(ko p) f -> p ko f", p=P))
                    for fi in range(fcw // P):
                        f = fc0 // P + fi
                        pq = ps_ca.tile([P, 512], F32, tag="pca", name="pcq")
                        for k in range(KC):
                            nc.tensor.matmul(pq[:, 0:R], cqc[:, k, bass.ts(fi, P)],
                                             x2T[:, k, :],
                                             start=(k == 0), stop=(k == KC - 1))
                        nc.scalar.activation(qcaT[:, f, :], pq[:, 0:R], AFT.Copy,
                                             scale=0.125)

            attnCT = cap.tile([P, HP, R], F32R, tag="attnCT")  # pair-form
            srec2 = dram.tile([HP, 2 * R], F32)
            estcs = []
            with tc.tile_pool(name="ps_cs", bufs=2, space="PSUM") as ps_cs, \
                 tc.tile_pool(name="ps_css", bufs=2, space="PSUM") as ps_css, \
                 tc.tile_pool(name="ps_cav", bufs=1, space="PSUM") as ps_cav, \
                 tc.tile_pool(name="expc", bufs=6) as expc:
                # pass 1: scores, exp, rowsum, reciprocal -> DRAM for all pairs
                for hp in range(HP):
                    estc = expc.tile([NTP, 2, R], F32R, tag="estc",
                                     name=f"estc{hp}")
                    estcs.append(estc)
                    nc.vector.memset(estc[:].bitcast(F32), 0.0)
                    for h01 in range(2):
                        b0 = h01 * D
                        psc = ps_cs.tile([P, R], F32, tag="pcs")
                        nc.tensor.matmul(psc[0:NTP, :], KcaT[b0:b0 + D, hp, :],
                                         qcaT[b0:b0 + D, hp, :],
                                         start=True, stop=True)
                        nc.scalar.activation(estc[0:NT, h01, :], psc[0:NT, :], AFT.Exp)
                    pss = ps_css.tile([1, 2 * R], F32, tag="pcss")
                    nc.tensor.matmul(pss[:], ones_r[0:NTP, :],
                                     estc[:, :, :].rearrange("p a b -> p (a b)"),
                                     start=True, stop=True)
                    rs = tmp.tile([1, 2 * R], F32, tag="rs")
                    nc.vector.reciprocal(rs[:], pss[:])
                    nc.sync.dma_start(srec2[hp:hp + 1, :], rs[:])
                # pass 2: broadcast reciprocals, attnV, divide
                for hp in range(HP):
                    estc = estcs[hp]
                    rbcA = tmp.tile([D, R], F32, tag="rbcA")
                    nc.sync.dma_start(rbcA[:],
                                      srec2[hp:hp + 1, 0:R].to_broadcast([D, R]))
                    rbcB = tmp.tile([D, R], F32, tag="rbcB")
                    nc.sync.dma_start(rbcB[:],
                                      srec2[hp:hp + 1, R:2 * R].to_broadcast([D, R]))
                    pavA = ps_cav.tile([D, R], F32, tag="pcavA")
                    nc.tensor.matmul(pavA[:],
                                     Vca[:, (2 * hp) * D:(2 * hp + 1) * D],
                                     estc[:, 0, :], start=True, stop=True)
                    pavB = ps_cav.tile([D, R], F32, tag="pcavB")
                    nc.tensor.matmul(pavB[:],
                                     Vca[:, (2 * hp + 1) * D:(2 * hp + 2) * D],
                                     estc[:, 1, :], start=True, stop=True)
                    nc.vector.tensor_mul(attnCT[0:D, hp, :], pavA[:], rbcA[:])
                    ost = tmp.tile([D, R], F32R, tag="ost")
                    nc.vector.tensor_mul(ost[:], pavB[:], rbcB[:])
                    nc.sync.dma_start(attnCT[D:P, hp, :], ost[:])

            # CA O-proj + bias + residual -> out
            outt = cap.tile([P, 2, C], F32, tag="outt")
            with tc.tile_pool(name="wstr4", bufs=2) as wstr4, \
                 tc.tile_pool(name="ps_co", bufs=2, space="PSUM") as ps_co:
                for f0, fw in ((0, 512), (512, 256)):
                    coc = wstr4.tile([P, HP, 512], F32R, tag="coc")
                    nc.gpsimd.dma_start(
                        coc[:, :, 0:fw],
                        co_d[:, f0:f0 + fw].rearrange("(hp p) f -> p hp f", p=P))
                    for qc in range(2):
                        po = ps_co.tile([P, 512], F32, tag="pco")
                        for hp in range(HP):
                            nc.tensor.matmul(po[:, 0:fw],
                                             attnCT[:, hp, bass.ts(qc, P)],
                                             coc[:, hp, 0:fw],
                                             start=(hp == 0), stop=(hp == HP - 1))
                        os_ = outt[:, qc, f0:f0 + fw]
                        nc.vector.tensor_add(os_, po[:, 0:fw], cobB[:, f0:f0 + fw])
                        nc.vector.tensor_add(os_, os_, x2[:, qc, f0:f0 + fw])

            nc.sync.dma_start(out_d.rearrange("(rc p) c -> p rc c", p=P), outt[:])

    nc.compile()
    return nc


def kernel(**inputs):
    if "nc" not in _cache:
        _cache["nc"] = build()
    nc = _cache["nc"]

    f32 = lambda a: np.ascontiguousarray(np.asarray(a), dtype=np.float32)
    hs = f32(inputs["hidden_states"])
    ehs = f32(inputs["encoder_hidden_states"])
    weights = {k: f32(inputs[k]) for k in (
        "sa_wq", "sa_wk", "sa_wv", "sa_wo", "sa_wo_b",
        "ln1_g", "ln1_b", "ln2_g", "ln2_b", "ff_ln_g", "ff_ln_b",
        "ff_w1", "ff_w2", "ca_wq", "ca_wk", "ca_wv", "ca_wo", "ca_wo_b")}
    aa = f32(inputs["alpha_attn"]).reshape(1, 1)
    ad = f32(inputs["alpha_dense"]).reshape(1, 1)

    in_maps = []
    for c in range(8):
        b, r = c // 4, c % 4
        m = dict(weights)
        m["x_own"] = np.ascontiguousarray(hs[b, r * R:(r + 1) * R])
        m["x_full"] = np.ascontiguousarray(hs[b])
        m["ehs"] = np.ascontiguousarray(ehs[b])
        m["alpha_attn"] = aa
        m["alpha_dense"] = ad
        in_maps.append(m)

    res = run_bass_kernel_spmd(nc, in_maps, core_ids=list(range(8)))
    _cache["last_res"] = res
    out = np.empty((B, N, C), np.float32)
    for c in range(8):
        b, r = c // 4, c % 4
        out[b, r * R:(r + 1) * R] = res.results[c]["out_own"]
    return out

